# revision 28
# baseline (speedup 1.0000x reference)
"""Linear (kernel-feature) attention for Trainium2, sharded over 8 NeuronCores.

Problem: B=4, H=16, S=4096, D=64 fp32.
    phi(x) = elu(x) + 1 = min(exp(x),1) + relu(x)
    kv   = (phi_k * mask)^T @ V        [d, v]
    k1   = sum_n phi_k * mask          [d]
    out  = (phi_q @ kv) / (phi_q @ k1 + eps)

Sharding: 64 (b,h) slices -> 8 per core. No cross-core communication.

All device data is bf16 (host converts; fp32 accumulate in PSUM). The mask is
folded into v on the host: v_ext[n, t, :] = [v[n]*mask[n] | mask[n]], so
kv_ext = phi_k^T @ v_ext yields [kv | k1] in one accumulating matmul chain
per n-tile and the device never touches the mask.

Host-side layout (part of sharding, costs no HW time). Pair-merged so every
DMA moves 8KB+ contiguous per partition:
  - qT:    [4 pairs, 128, 4096]   bf16, q transposed per slice ([d, n]), two
           slices stacked on the partition dim (M2 contracts over d).
  - kc:    [4 pairs, 128, 2*2048] bf16 partition-tiled natural layout.
  - vc:    [4 pairs, 128, 2*2080] bf16 v_ext layout (65th col per tile=mask).
  - outc:  [4 pairs, 128, 2*2048] bf16.

phi is computed as min(exp(x), 1 + relu(x)) — identical to min(exp(x),1) +
relu(x) for all x (for x>0, exp(x) > 1+x so the min picks 1+x; for x<=0,
relu=0 and exp<=1) — which needs no ACT relu pass: r1 = (x max 0) add 1 is a
2-op tensor_scalar (4x) and the combine is a tensor_tensor min (2x bf16).
q is pre-scaled by 1/sqrt(D)=2^-3 on the host (exact exponent shift, the
bf16 values are bit-identical in relative precision).

Engine split per pair (phi passes are [128, 4096] each):
  ACT: exp_k, exp_q                               (1 elem/cyc/part @1.2GHz)
  DVE: r1 ts (4x), phi_q tt-min (2x bf16), kv evac, normalizer reciprocal,
       divide-multiply (1x, PSUM operand)
  Pool: phi_k tt-min (k-side combine offloaded; Pool is otherwise idle)
  PE:  M1 32 accumulating matmuls K=128 rhs=65 cols per slice (two slices in
       PSUM partition halves); M2 lhsT=phi_qT[64,128] rhs=kv_ext[64,65], 7
       n-tiles per 512-col PSUM sub-bank, normalizer rides as col 64; the
       divide reads two banks per DVE op.
"""

import sys

sys.path.insert(0, "/opt/trn_rl_repo")

import numpy as np

B, H, S, D = 4, 16, 4096, 64
N_CORES = 8
SL = (B * H) // N_CORES  # slices per core = 8
PAIRS = SL // 2  # 4
NT = S // 128  # 32 n-tiles per slice
FREE = NT * D  # 2048 free cols for k/out slice layout
VFREE = NT * (D + 1)  # 2080 free cols for v_ext
GROUPS = [(0, 14), (14, 14), (28, 4)]  # (tile0, ntiles) per 2-bank PSUM tile
EPS = 1e-6  # absorbed: normalizer ~3e5, eps drop changes result by ~3e-12

# which engine runs the k-side phi combine. gpsimd would balance load, but
# walrus rejects TensorTensor/TensorScalar opcodes on Pool for core v3.
K_MIN_ENGINE = "vector"

_programs: dict = {}


def _build_program(reps: int = 1):
    from contextlib import ExitStack

    import concourse.bacc as bacc
    import concourse.tile as tile
    from concourse import mybir

    f32 = mybir.dt.float32
    bf16 = mybir.dt.bfloat16
    Alu = mybir.AluOpType
    Act = mybir.ActivationFunctionType

    nc = bacc.Bacc("TRN2", target_bir_lowering=False, debug=False)
    qT = nc.dram_tensor("qT", [PAIRS, 128, S], bf16, kind="ExternalInput").ap()
    kc = nc.dram_tensor("kc", [PAIRS, 128, 2 * FREE], bf16, kind="ExternalInput").ap()
    vc = nc.dram_tensor("vc", [PAIRS, 128, 2 * VFREE], bf16, kind="ExternalInput").ap()
    outc = nc.dram_tensor(
        "outc", [PAIRS, 128, 2 * FREE], bf16, kind="ExternalOutput"
    ).ap()

    with tile.TileContext(nc) as tc, ExitStack() as ctx:
        kqp = ctx.enter_context(tc.tile_pool(name="kqp", bufs=4))
        vp = ctx.enter_context(tc.tile_pool(name="vp", bufs=4))
        tmp = ctx.enter_context(tc.tile_pool(name="tmp", bufs=2))
        kvp = ctx.enter_context(tc.tile_pool(name="kvp", bufs=2))
        nrmp = ctx.enter_context(tc.tile_pool(name="nrmp", bufs=6))
        outp = ctx.enter_context(tc.tile_pool(name="outp", bufs=2))
        ps_kv = ctx.enter_context(tc.tile_pool(name="ps_kv", bufs=2, space="PSUM"))
        ps_out = ctx.enter_context(tc.tile_pool(name="ps_out", bufs=3, space="PSUM"))

        k_min = nc.vector if K_MIN_ENGINE == "vector" else nc.gpsimd

        for _rep in range(reps):
            for pair in range(PAIRS):
                # ---- load K (cols 0:2*FREE) and qT (cols 2*FREE:) into one
                # combined tile; phi for all of it in one ts + one tt pass.
                # exp stays split so M1 need not wait for the q half.
                kt = kqp.tile([128, 2 * FREE + S], bf16)
                qt = kt[:, 2 * FREE : 2 * FREE + S]
                nc.sync.dma_start(out=kt[:, 0 : 2 * FREE], in_=kc[pair])
                nc.sync.dma_start(out=qt, in_=qT[pair])
                vt = vp.tile([128, 2 * VFREE], bf16)
                nc.sync.dma_start(out=vt, in_=vc[pair])
                # phi split at the k/q boundary so M1 need not wait for the
                # q DMA: the k chain (ts_r1 needs only the DMA, not exp) can
                # start as soon as kc lands.
                e = tmp.tile([128, 2 * FREE + S], bf16, tag="e")
                rl = tmp.tile([128, 2 * FREE + S], bf16, tag="r")
                for c0, c1 in ((0, FREE), (FREE, 2 * FREE), (2 * FREE, 2 * FREE + S)):
                    nc.vector.tensor_scalar(
                        rl[:, c0:c1], kt[:, c0:c1], 0.0, 1.0, Alu.max, Alu.add
                    )
                    nc.scalar.activation(e[:, c0:c1], kt[:, c0:c1], Act.Exp)
                    k_min.tensor_tensor(
                        kt[:, c0:c1], e[:, c0:c1], rl[:, c0:c1], Alu.min
                    )

                # ---- M1: kv_ext[64,65] per slice, packed into PSUM halves.
                kv_ps = ps_kv.tile([128, 512], f32)
                for t in range(NT):
                    st, sp = (t == 0), (t == NT - 1)
                    for r in range(2):
                        nc.tensor.matmul(
                            kv_ps[64 * r : 64 * r + 64, 0:65],
                            kt[:, r * FREE + t * D : r * FREE + (t + 1) * D],
                            vt[:, r * VFREE + t * (D + 1) : r * VFREE + (t + 1) * (D + 1)],
                            start=st,
                            stop=sp,
                            skip_group_check=True,
                        )
                kv_sb = kvp.tile([128, 65], bf16)
                nc.scalar.copy(kv_sb, kv_ps[:, 0:65])

                # ---- M2 + divide + store per slice; pair shares one out tile.
                # po is a 2-bank PSUM tile; 7 n-tiles of 65 cols per 512-col
                # sub-bank (matmul groups never cross a bank; start=True on
                # the first matmul touching each sub-bank clears has_written).
                out_sb = outp.tile([128, 2 * FREE], bf16)
                for r in range(2):
                    rhs_ext = kv_sb[64 * r : 64 * r + 64, 0:65]
                    ob = r * FREE
                    outc_half = outc[pair][:, ob : ob + FREE]
                    for t0, gn in GROUPS:
                        po = ps_out.tile([128, 1024], f32)
                        for i in range(gn):
                            t = t0 + i
                            col = (i // 7) * 512 + (i % 7) * 65
                            nc.tensor.matmul(
                                po[:, col : col + 65],
                                qt[64 * r : 64 * r + 64, t * 128 : (t + 1) * 128],
                                rhs_ext,
                                start=(i % 7 == 0),
                                stop=(i == gn - 1 or i % 7 == 6),
                                skip_group_check=True,
                            )
                        nb, gi = (gn + 6) // 7, min(gn, 7)  # sub-banks, tiles/bank
                        pg = (
                            po.rearrange("p (c x) -> p c x", c=2)[:, 0:nb, 0 : gi * 65]
                            .rearrange("p c (a b) -> p c a b", a=gi)
                        )
                        nsb = nrmp.tile([128, 16], f32)
                        nr = nsb[:, 0 : nb * gi].rearrange(
                            "p (c a) -> p c a", c=nb
                        )
                        nc.vector.reciprocal(
                            nr.rearrange("p c (a b) -> p c a b", b=1),
                            pg[:, :, :, 64:65],
                        )
                        nc.vector.tensor_tensor(
                            out_sb[:, ob + t0 * D : ob + (t0 + gn) * D].rearrange(
                                "p (c a b) -> p c a b", c=nb, a=gi
                            ),
                            pg[:, :, :, 0:64],
                            nr.rearrange("p c (a b) -> p c a b", b=1).broadcast_to(
                                [128, nb, gi, 64]
                            ),
                            Alu.mult,
                        )
                    # store per slice: shortens the tail after the last divide
                    nc.sync.dma_start(
                        out=outc_half, in_=out_sb[:, ob : ob + FREE]
                    )

    nc.compile()
    return nc


def _get_program(reps: int = 1):
    if reps not in _programs:
        _programs[reps] = _build_program(reps)
    return _programs[reps]


def _pack_inputs(query, key, value, attention_mask):
    """Shard + lay out + bf16-convert inputs for the 8 cores."""
    from concourse import mybir

    bf16 = mybir.dt.np(mybir.dt.bfloat16)

    q4 = np.asarray(query, dtype=np.float32).reshape(B * H, S, D)
    k4 = np.asarray(key, dtype=np.float32).reshape(B * H, S, D)
    v4 = np.asarray(value, dtype=np.float32).reshape(B * H, S, D)
    am = np.asarray(attention_mask, dtype=np.float32)

    # qT: [g, d, n] -> per core [PAIRS, 128, S]; pre-scaled by 1/sqrt(D)=2^-3
    # (exact exponent shift — bit-identical relative precision in bf16)
    qT = (
        np.ascontiguousarray(q4.transpose(0, 2, 1) * np.float32(0.125))
        .reshape(N_CORES, PAIRS, 128, S)
        .astype(bf16)
    )
    # kc: [g, t, p, d] -> [g, p, t*d], pairs merged on the last axis
    kcl = (
        np.ascontiguousarray(k4.reshape(B * H, NT, 128, D).transpose(0, 2, 1, 3))
        .reshape(N_CORES, PAIRS, 2, 128, FREE)
        .transpose(0, 1, 3, 2, 4)
        .reshape(N_CORES, PAIRS, 128, 2 * FREE)
    )
    kcl = np.ascontiguousarray(kcl).astype(bf16)
    # v_ext: [g, p, t, d+1] with col d = mask, v pre-multiplied by mask
    vp_ = v4.reshape(B * H, NT, 128, D).transpose(0, 2, 1, 3)  # [g, p, t, d]
    am_pt = (
        am[np.arange(B * H) // H]  # [g, S]
        .reshape(B * H, NT, 128)
        .transpose(0, 2, 1)  # [g, p, t]
    )
    vext = np.concatenate([vp_ * am_pt[..., None], am_pt[..., None]], axis=3)
    vcl = (
        vext.reshape(N_CORES, PAIRS, 2, 128, VFREE)
        .transpose(0, 1, 3, 2, 4)
        .reshape(N_CORES, PAIRS, 128, 2 * VFREE)
    )
    vcl = np.ascontiguousarray(vcl).astype(bf16)

    in_maps = [{"qT": qT[c], "kc": kcl[c], "vc": vcl[c]} for c in range(N_CORES)]
    return in_maps, False


def _unpack_output(results):
    outs = np.stack([np.asarray(r["outc"], dtype=np.float32) for r in results])
    # [cores, PAIRS, 128, 2*FREE] -> [g, 128, t, d]
    outs = outs.reshape(B * H // 2, 128, 2, NT, D).transpose(0, 2, 1, 3, 4)
    outs = outs.reshape(B * H, 128, NT, D).transpose(0, 2, 1, 3)  # [g, t, p, d]
    return np.ascontiguousarray(outs).reshape(B, H, S, D)


def kernel(query, key, value, attention_mask):
    from concourse.bass_utils import run_bass_kernel_spmd

    in_maps, _ = _pack_inputs(query, key, value, attention_mask)
    nc = _get_program()
    res = run_bass_kernel_spmd(nc, in_maps, core_ids=list(range(N_CORES)))
    return _unpack_output(res.results)
